# revision 18
# baseline (speedup 1.0000x reference)
"""Trainium2 Bass kernel for nn_BiaffineModule (biaffine span scorer).

Math (reference):
    x  = concat(final_hidden, feature_vecs)        [B,S,H+F]
    s  = x @ start_W + start_b                     [B,S,T]
    e  = x @ end_W + end_b                         [B,S,T]
    pre[b,s,e,c]  = sum_u (s @ U[:,c,:])[b,s,u] * e[b,e,u]
    ffn[b,s,e,c]  = (s@Ws)[b,s,c] + (e@We)[b,e,c] + (wh@Ww)[s,e,c] + lin_b[c]
    out = pre + ffn                                [B,S,S,C]

Sharding: the start axis `s` is split 8 ways (32 rows per core); each core
computes the full [B, 32, S, C] slab of the pairwise grid for all batches.
Small params + x are replicated; width_hidden and the x slab-columns are
sliced per core.

All large matmul operands are bf16 (1 cycle/row on the PE regardless of
free size, half the HBM bytes of f32); the width term is fp8e4m3 (it is
~1% of output magnitude, so fp8's ~4% rel err contributes ~4e-4).
Output is stored bf16 and upcast on the host. End-to-end rel err vs the
fp32 reference ~5e-3 (gate is 2e-2).

Every DRAM input is host-packed into the exact SBUF tile layout
([partitions, per-partition words]) and fused into few large DMAs —
many small issues serialize on the ~0.65us HWDGE descriptor-gen cost
and on DMA-semaphore recycling.

Per-core dataflow (contraction dims on SBUF partitions):
    sT   [t,row]     = sW_aug^T @ x-slab-cols              (bf16)
    eT   [t,(b,e)]   = eW_aug^T @ xT, per b-pair halves    (bf16)
    sUT  [u,(c,row)] = sum_t U[t,c,u] sT[t,row]            (bf16)
    fsT  [c,row]     = Ws^T @ sT + lin_b x ones            (K=1 fold)
    fw   [e,(c,s)]   = sum_w whT[w,e] Ww[w,c]              (fp8)
    fe   [e,c]       = We^T-fold of eT                     (bf16)
    planes[b,ech]    = fw + fe-bcast                       (GpSimd, early)
    out[e,(c,s)] per (b,e-chunk), PSUM-accumulated:
        2 MMs biaffine (eT x sUT) + 1 K=1 MM (ones x fs-row),
    then one DVE add (ps + planes) and a bf16 store.

PE emission order: warmup (clock ramp), sT, fsT, fw (fills the wait
for U), sUT groups, eT halves + fe, then the 8 stage5 blocks.
Loads: the fast sync DMA queue (~260GB/s vs the scalar queue's
~115-160) carries sx -> U halves -> xT halves in consumption order,
each queue's first-use spin-up paid by a tiny dummy DMA; linPack
rides inside the sx pack (a separate tiny DMA's completion semaphore
fires microseconds late). Stores alternate between the two rings.
Accumulation chains stay serial per PSUM tile - interleaving chains
across PSUM tiles slows the PE (group switching beats ldweights
overlap).

Host unshards results[k][b,e,(c,s)] -> full[b, k*32+s, e, c].
"""

import sys

import numpy as np

sys.path.insert(0, "/opt/trn_rl_repo")

B, S, H, F = 4, 256, 768, 32
T, WD, C = 256, 64, 16
NCORES = 8
SLAB = S // NCORES          # 32 s-rows per core
ROWS = B * SLAB             # 128 slab rows (b-major, s-minor)
NB = B * S                  # 1024 xT columns (b-major, s-minor)
KPAD = 896                  # 7 * 128 (zero-padded H+F+1 ones-row)
NKCH = KPAD // 128

_CACHE = {}


def _build():
    import concourse.bacc as bacc
    import concourse.mybir as mybir
    from concourse import tile

    f32 = mybir.dt.float32
    bf16 = mybir.dt.bfloat16
    f8 = mybir.dt.float8e4

    nc = bacc.Bacc(
        "TRN2", target_bir_lowering=False, debug=False, num_devices=NCORES
    )

    # host-packed to SBUF layout; sx = xsl|sW fused, wl = whT|linWw fused
    sx_d = nc.dram_tensor("sxp", [128, NKCH * (ROWS + T) + 5 * C], bf16, kind="ExternalInput")
    eW_d = nc.dram_tensor("eWp", [128, NKCH * T], bf16, kind="ExternalInput")
    xT_d = nc.dram_tensor("xTp", [128, B * NKCH * 256], bf16, kind="ExternalInput")
    U_d = nc.dram_tensor("Up", [128, 4 * 2 * 1024], bf16, kind="ExternalInput")
    wl_d = nc.dram_tensor("wlp", [WD, SLAB * S + C], f8, kind="ExternalInput")
    lp_d = nc.dram_tensor("linPack", [128, 5 * C], bf16, kind="ExternalInput")
    out_d = nc.dram_tensor("out", [B, S, SLAB * C], bf16, kind="ExternalOutput")

    with tile.TileContext(nc) as tc:
        with (
            tc.tile_pool(name="consts", bufs=1) as consts,
            tc.tile_pool(name="acts", bufs=1) as acts,
            tc.tile_pool(name="outp", bufs=3) as outp,
            tc.tile_pool(name="pmm", bufs=2, space="PSUM") as pmm,
            tc.tile_pool(name="pmm2", bufs=2, space="PSUM") as pmm2,
            tc.tile_pool(name="ps5", bufs=3, space="PSUM") as ps5,
            tc.tile_pool(name="pfw", bufs=1, space="PSUM") as pfw,
        ):
            ones1 = consts.tile([1, 128], bf16)
            nc.vector.memset(ones1[:], 1.0)

            # ---- loads ---------------------------------------------------
            # scalar queue spins up ~2us before sync: urgent small loads
            # (sx for sT, lp, wl for fw, eW) go there; sync carries the
            # bulk U quarters + xT halves.
            sxb2 = consts.tile([128, NKCH * (ROWS + T) + 5 * C], bf16)
            sxb = sxb2[:, 0 : NKCH * (ROWS + T)].rearrange(
                "p (n w) -> p n w", n=NKCH
            )
            lpb = sxb2[:, NKCH * (ROWS + T) :].rearrange("p (n c) -> p n c", c=C)
            wlb = consts.tile([WD, SLAB * S + C], f8)
            eWb = consts.tile([128, NKCH, T], bf16)
            # The sync-ring DMA queue sustains ~250GB/s, the scalar ring
            # ~115GB/s; aggregate ~400. Critical-path loads (sx, U, xT) on
            # sync in consumption order; wl + eW on scalar. 8 load DMAs
            # total = no semaphore-recycling stalls.
            Ub = consts.tile([128, 4, 2, 1024], bf16)
            Uv = U_d.ap().rearrange("p (g n w) -> p g n w", g=4, n=2)
            xTb = consts.tile([128, B, NKCH, 256], bf16)
            xTv = xT_d.ap().rearrange("p (b n w) -> p b n w", b=B, n=NKCH)
            # three DMA queues (sync/scalar/gpsimd), each paying its
            # spin-up on a tiny dummy first. U alone on sync (its arrival
            # heads the critical chain); wl+sx+xT on gpsimd; small stuff
            # on the slow scalar queue.
            dummy = consts.tile([1, 2, C], bf16)
            nc.sync.dma_start(dummy[:, 0, :], lp_d[0:1, 0:C])
            nc.scalar.dma_start(dummy[:, 1, :], lp_d[0:1, C : 2 * C])
            # fast sync queue: sx(+lp) -> U -> xT in consumption order
            nc.sync.dma_start(sxb2[:], sx_d[:])
            nc.sync.dma_start(Ub[:, 0:2], Uv[:, 0:2])
            nc.sync.dma_start(Ub[:, 2:4], Uv[:, 2:4])
            nc.sync.dma_start(xTb[:, 0:2], xTv[:, 0:2])
            nc.sync.dma_start(xTb[:, 2:4], xTv[:, 2:4])
            # slow scalar queue: the small fw/eT weights
            nc.scalar.dma_start(wlb[:], wl_d[:])
            nc.scalar.dma_start(eWb[:], eW_d.ap().rearrange("p (n t) -> p n t", n=NKCH))
            whb = wlb[:, 0 : SLAB * S]
            lwb = wlb[:, SLAB * S : SLAB * S + C]

            # ---- activations ---------------------------------------------
            sT = acts.tile([128, 2, ROWS], bf16)
            eT = acts.tile([128, 2, NB], bf16)
            sUT = [acts.tile([128, C, ROWS], bf16, name=f"sUT{u}") for u in range(2)]
            fw = acts.tile([128, 2, C, SLAB], bf16)
            fsT = acts.tile([16, ROWS], bf16)
            fsr = acts.tile([1, B, SLAB * C], bf16)
            feS = acts.tile([128, B, 2, C], bf16)
            planes = acts.tile([128, B, 2, C, SLAB], bf16)

            # PE warmup: ramp the clock while loads stream (no input deps)
            wps = pmm.tile([128, 128], f32, tag="pmm", name="warm")
            for _ in range(24):
                nc.tensor.matmul(wps[:], ones1[:], ones1[:], start=True, stop=True)

            def sT_stage():
                for tch in range(2):
                    ps = pmm.tile([128, ROWS], f32, tag="pmm")
                    for k in range(NKCH):
                        nc.tensor.matmul(
                            ps[:],
                            sxb[:, k, ROWS + tch * 128 : ROWS + (tch + 1) * 128],
                            sxb[:, k, 0:ROWS],
                            start=(k == 0),
                            stop=(k == NKCH - 1),
                        )
                    nc.vector.tensor_copy(sT[:, tch, :], ps[:])

            def fsT_stage():
                psf = pmm.tile([16, ROWS], f32, tag="pmm")
                for tch in range(2):
                    nc.tensor.matmul(
                        psf[:], lpb[:, tch, :], sT[:, tch, :],
                        start=(tch == 0), stop=False,
                    )
                nc.tensor.matmul(
                    psf[:], lpb[0:1, 4, :], ones1[:], start=False, stop=True
                )
                nc.vector.tensor_copy(fsT[:], psf[:])
                for b in range(B):
                    nc.scalar.dma_start(
                        fsr[0:1, b, :], fsT[:, b * SLAB : (b + 1) * SLAB]
                    )

            def fw_stage(ech):
                ps = pfw.tile([128, SLAB, C], f32, tag="pfw")
                for s in range(SLAB):
                    nc.tensor.matmul(
                        ps[:, s, :],
                        whb[:, s * S + ech * 128 : s * S + ech * 128 + 128],
                        lwb,
                        start=True,
                        stop=True,
                    )
                src = ps[:]
                src = type(src)(
                    src.tensor, src.offset, [src.ap[0], [1, C], [C, SLAB]]
                )
                nc.vector.tensor_copy(fw[:, ech, :, :], src)

            def sUT_group(grp):
                for uch in range(2):
                    ps = pmm.tile([128, 512], f32, tag="pmm")
                    for cl in range(4):
                        for tch in range(2):
                            nc.tensor.matmul(
                                ps[:, cl * 128 : (cl + 1) * 128],
                                Ub[:, grp, tch, cl * 256 + uch * 128 : cl * 256 + uch * 128 + 128],
                                sT[:, tch, :],
                                start=(tch == 0),
                                stop=(tch == 1),
                            )
                    dst = sUT[uch][:, grp * 4 : (grp + 1) * 4, :]
                    if (grp + uch) % 2 == 0:
                        nc.vector.tensor_copy(dst, ps[:])
                    else:
                        nc.scalar.copy(dst, ps[:])

            def eT_half(h):
                # b-pair (2h, 2h+1); rhs free = [2 batches, 256 cols]
                for tch in range(2):
                    ps = pmm2.tile([128, 512], f32, tag="pmm2")
                    for k in range(NKCH):
                        nc.tensor.matmul(
                            ps[:],
                            eWb[:, k, tch * 128 : (tch + 1) * 128],
                            xTb[:, 2 * h : 2 * h + 2, k, :],
                            start=(k == 0),
                            stop=(k == NKCH - 1),
                        )
                    nc.vector.tensor_copy(
                        eT[:, tch, h * 512 : (h + 1) * 512], ps[:]
                    )

            def fe_stage(b):
                for ech in range(2):
                    ecols = slice(b * S + ech * 128, b * S + ech * 128 + 128)
                    psq = pmm.tile([128, 16], f32, tag="pmm", name=f"feq{b}{ech}")
                    for tch in range(2):
                        nc.tensor.matmul(
                            psq[:],
                            eT[:, tch, ecols],
                            lpb[:, 2 + tch, :],
                            start=(tch == 0),
                            stop=(tch == 1),
                        )
                    nc.scalar.copy(feS[:, b, ech, :], psq[:])
                    # planes = fw + fe-bcast on GpSimd, off the critical tail
                    feb = feS[:, b, ech, :]
                    feb = type(feb)(
                        feb.tensor, feb.offset, [feb.ap[0], [1, C], [0, SLAB]]
                    )
                    nc.gpsimd.tensor_add(
                        planes[:, b, ech, :, :], fw[:, ech, :, :], feb
                    )

            def stage5(b):
                for ech in range(2):
                    ps = ps5.tile([128, SLAB * C], f32, tag="ps5")
                    ecols = slice(b * S + ech * 128, b * S + ech * 128 + 128)
                    for uch in range(2):
                        nc.tensor.matmul(
                            ps[:],
                            eT[:, uch, ecols],
                            sUT[uch][:, :, b * SLAB : (b + 1) * SLAB],
                            start=(uch == 0),
                            stop=False,
                        )
                    nc.tensor.matmul(
                        ps[:], ones1[:], fsr[0:1, b, :], start=False, stop=True
                    )
                    ob = outp.tile([128, SLAB * C], bf16, tag="outp")
                    nc.vector.tensor_add(
                        ob[:].rearrange("p (c s) -> p c s", c=C),
                        ps[:].rearrange("p (c s) -> p c s", c=C),
                        planes[:, b, ech, :, :],
                    )
                    eng = nc.sync if (b * 2 + ech) % 2 == 0 else nc.scalar
                    eng.dma_start(
                        out_d[b, ech * 128 : (ech + 1) * 128, :], ob[:]
                    )

            # emission order matched to DMA arrival order above
            sT_stage()
            fsT_stage()
            fw_stage(0)
            fw_stage(1)
            sUT_group(0)
            sUT_group(1)
            sUT_group(2)
            sUT_group(3)
            for h in range(2):
                eT_half(h)
                fe_stage(2 * h)
                fe_stage(2 * h + 1)
            for b in range(B):
                stage5(b)

    nc.compile()
    return nc


def _get_nc():
    if "nc" not in _CACHE:
        _CACHE["nc"] = _build()
    return _CACHE["nc"]


def _pack_kchunks(Wa):
    """[KPAD, w] -> [128, NKCH*w] in SBUF layout (partition p holds rows
    p, 128+p, ..., concatenated)."""
    w = Wa.shape[1]
    return np.ascontiguousarray(
        Wa.reshape(NKCH, 128, w).transpose(1, 0, 2).reshape(128, NKCH * w)
    )


def kernel(
    final_hidden, feature_vecs, start_W, start_b, end_W, end_b, U,
    width_hidden, lin_W, lin_b,
):
    import ml_dtypes

    from concourse.bass_utils import run_bass_kernel_spmd

    f32 = np.float32
    bf16 = ml_dtypes.bfloat16
    f8 = ml_dtypes.float8_e4m3
    fh = np.asarray(final_hidden, f32)
    fv = np.asarray(feature_vecs, f32)

    x = np.concatenate([fh, fv], axis=-1)                  # [B,S,H+F]
    xT = np.zeros((KPAD, NB), f32)
    xT[: H + F] = x.reshape(NB, H + F).T
    xT[H + F] = 1.0                                        # bias fold row
    xTa = xT.astype(bf16)
    # xTp[p, b, n, w] = xT[n*128+p, b*256+w]
    xTp = np.ascontiguousarray(
        xTa.reshape(NKCH, 128, B, 256).transpose(1, 2, 0, 3).reshape(128, -1)
    )

    def aug(W, bvec):
        Wa = np.zeros((KPAD, T), f32)
        Wa[: H + F] = np.asarray(W, f32)
        Wa[H + F] = np.asarray(bvec, f32)
        return Wa.astype(bf16)

    sWa = aug(start_W, start_b)
    eWp = _pack_kchunks(aug(end_W, end_b))
    U2 = np.asarray(U, f32).reshape(T, C * T).astype(bf16)
    # Up[p, g, n, w] = U2[n*128+p, g*1024+w]
    Up = np.ascontiguousarray(
        U2.reshape(2, 128, 4, 1024).transpose(1, 2, 0, 3).reshape(128, -1)
    )
    linW = np.asarray(lin_W, f32)
    linWw = linW[2 * T :].astype(f8)                       # [64, 16]
    linPack = np.zeros((128, 5 * C), f32)
    linPack[:, 0:C] = linW[0:128, :]
    linPack[:, C : 2 * C] = linW[128:256, :]
    linPack[:, 2 * C : 3 * C] = linW[T : T + 128, :]
    linPack[:, 3 * C : 4 * C] = linW[T + 128 : 2 * T, :]
    linPack[0, 4 * C : 5 * C] = np.asarray(lin_b, f32)
    linPack = linPack.astype(bf16)
    wh = np.asarray(width_hidden, f32)

    in_maps = []
    for k in range(NCORES):
        slab = wh[k * SLAB : (k + 1) * SLAB]               # [32, 256, 64]
        whT2 = slab.transpose(2, 0, 1).reshape(WD, SLAB * S).astype(f8)
        wlp = np.ascontiguousarray(np.concatenate([whT2, linWw], axis=1))
        cols = (
            np.arange(B)[:, None] * S + (k * SLAB + np.arange(SLAB))[None, :]
        ).reshape(-1)
        # sx: per k-chunk, xsl block then sW block
        xsl3 = xTa[:, cols].reshape(NKCH, 128, ROWS)
        sW3 = sWa.reshape(NKCH, 128, T)
        sxp = np.ascontiguousarray(
            np.concatenate(
                [
                    np.concatenate([xsl3, sW3], axis=2)
                    .transpose(1, 0, 2)
                    .reshape(128, -1),
                    linPack,
                ],
                axis=1,
            )
        )
        in_maps.append(
            {
                "xTp": xTp, "sxp": sxp, "eWp": eWp, "Up": Up,
                "wlp": wlp, "linPack": linPack,
            }
        )

    _CACHE["last_in_maps"] = in_maps
    nc = _get_nc()
    res = run_bass_kernel_spmd(nc, in_maps, core_ids=list(range(NCORES)))

    full = np.empty((B, S, S, C), f32)
    for k in range(NCORES):
        r = res.results[k]["out"].astype(f32).reshape(B, S, C, SLAB)
        full[:, k * SLAB : (k + 1) * SLAB] = r.transpose(0, 3, 1, 2)
    return full
